# revision 27
# baseline (speedup 1.0000x reference)
"""Windowed attention w/ ring-buffer KV cache for TRN2, 8 NeuronCores.

Problem (hardcoded): B=1, S=1024 new tokens, H=16 heads, D=64,
cache C=10240, window W=8192, START_FRAME=9728.

Math (derived from the reference ring-buffer update; the updated cache is
not returned, only the attention output):
  wk = concat(cache_k[2560:9728], rope(k)),  wv = concat(cache_v[2560:9728], v)
  out = softmax(rope(q) @ wk^T / 8) @ wv     (non-causal, all 8192 keys)

Sharding: head-parallel, 2 heads per core (core c owns heads 2c, 2c+1).
Each core computes its full [1024, 2, 64] output slice; host concatenates.

ScalarE exp (16.8M exps/core at 1 elem/cycle/lane ~ 110us floor) and the
PE matmul stream (~512 MMs x ~244ns measured in situ ~ 125us) are jointly
the bottleneck; the kernel keeps both saturated:
  - KT  [128, 8192] bf16: K^T, partitions = (head(2) x d(64)), cols = pos
  - QT  [128, 1024] bf16: Q^T, same partition layout
  - Vp  [128, 64kb, 128] bf16 per head: V natural layout; col 64 = 1.0
    (softmax denominator via the PV matmul) and cols 65:127 = 0 pad are
    BAKED HOST-SIDE into v_all, so no on-device memsets/ones-copies ever
    block the startup path, and the 128 non-fp32 weight cols keep FWL
    (fast weight load) on for PV LDWEIGHTS
  - rope: host ships q/k pre-transposed as [hd, tok] plus a pair-swapped
    copy and a sign-baked cos/sin table (all pure data movement, bf16),
    so rope(x)^T = xT*cosT + xTswap*sinTpm -- three DVE ops straight
    into QT/KT, no PE transposes, no PSUM traffic. The 6 DVE rope ops of
    the NEXT body are returned as closures and injected one-at-a-time
    between attention batches so the DVE FIFO never backs up.
  - QK  : S^T[k,q] via row-tiled bf16 matmuls (head0 rows 0-63, head1
    64-127); adjacent opposite-band MMs run CONCURRENTLY on the PE
    (HW-measured 128 ns/MM alternating vs 473 same-band)
  - exp : ScalarE over 3-bank PSUM batches (scale=1/8 fused), bf16 out;
    QK(b+DEPTH) is emitted after exp(b) so the bank-recycle chain
    exp(b) -> QK(b+DEPTH) -> exp(b+DEPTH) stays off the critical path
  - PV  : accumulate V'^T @ P^T into PSUM [128, 512] per (head, q-tile);
    row 64 = softmax denominator (from the ones column), rows 65-127
    are pad output and never read
  - tail: normalize in [d, tok] layout (reciprocal on DVE, denominator
    partition-broadcast on the idle GpSimd engine, multiply, DMA); the
    host transposes the output back to [tok, d]
  - startup (single-shot): the q halves of qk_new and KT chunk 0 are
    DMA'd before the k halves, so rope-q -> first QK batches begin ~11us
    in instead of ~22us.

A DVE fast-exp offload (Schraudolph int16-bits-as-bf16, FA/FB/_carve
below) is implemented and numerically validated (rel err ~2% rms on
carved slices, end-to-end ~7e-3 vs the 2e-2 gate) but DISABLED
(DVE_PAT empty): on HW, every carved variant measured slower
(127-150us/iter vs 124.7) because the PE matmul stream with per-MM sem
overhead is itself at ~125us/body, so offloading ScalarE work only adds
cross-engine coupling without lowering the wall. It becomes profitable
only if PV moves to fp8 DoubleRow (halving PV PE time) -- untested.

Steady-state pipelining (loop mode): input tiles are double-buffered by
body parity; each body emits the next body's DMAs + rope up front so they
stream during this body's attention, and pre-emits the next body's first
QK batches so ScalarE never idles at the body boundary.
"""

import numpy as np

H, D = 16, 64
S = 1024
W = 8192
OLD = 7168          # window rows taken from old cache (cache rows 2560:9728)
CLO, CHI = 2560, 9728
START = 9728
NCORES = 8
NKB = W // 128      # 64
NQT = S // 128      # 8 token blocks
SCALE = 0.125
UNROLL = 20         # bodies per For_i iteration (amortizes the barrier)

# DVE fast-exp (Schraudolph in bf16 bit-space): for a carved slice the DVE
# computes int16(rint(x*FA + FB)) whose bits, read as bf16, are ~exp(x/8)
# (rel err ~2% rms, zero-mean; softmax averaging washes it out). One DVE
# tensor_scalar per carved slice vs one ScalarE exp element-slot — this
# offloads ~35% of the exp work to the otherwise-idle Vector engine.
FA = 128.0 * float(np.log2(np.e)) / 8.0   # folds the 1/8 score scale
FB = 127.0 * 128.0 - 7.25                 # exponent bias + sawtooth centering
BATCH = 3           # slices per st tile (3 PSUM banks)
DEPTH = 2           # st rotation depth: QK(b+DEPTH) overlaps exp/carve(b)
# Whole-batch carve pattern: a batch is processed entirely by ScalarE (true
# exp) or entirely by the DVE (fast-exp). Whole batches mean one instruction
# per batch per engine and halve each engine's bank-recycle cadence, hiding
# cross-engine semaphore latency. 5/12 of batches -> DVE balances
# ScalarE ~77us vs DVE ~75us per body.
DVE_PAT = frozenset()
DVE_MOD = 5
PVD = 0             # PV trails the exp stream by this many batches

_cache = {}


def _carve(b, nb):
    """Slices of batch b the DVE takes: all of them or none."""
    return nb if b % DVE_MOD in DVE_PAT else 0


def _build(niters=1, loop=False):
    import concourse.mybir as mybir
    import concourse.tile as tile
    from concourse import bacc
    from concourse._compat import axon_active
    from concourse.bass import ds

    dt = mybir.dt.float32
    bt = mybir.dt.bfloat16
    AF = mybir.ActivationFunctionType
    ALU = mybir.AluOpType

    nc = bacc.Bacc(
        "TRN2", target_bir_lowering=False, debug=not axon_active(),
        num_devices=NCORES,
    )
    kt_old = nc.dram_tensor("kt_old", [128, OLD], bt, kind="ExternalInput")
    # v_all carries the full 128 weight cols per head: cols 0:64 = V, col 64
    # = 1.0 (softmax denominator via the PV matmul), 65:128 = 0 pad (FWL
    # needs 128 non-fp32 weight cols). Baked host-side so no on-device
    # memsets/ones-copies are needed.
    v_all = nc.dram_tensor("v_all", [2, 128, NKB, 128], bt, kind="ExternalInput")
    # qk_new: [qT, qTswap, kT, kTswap], each [128=(h d), 1024 tok] fp32
    qk_new = nc.dram_tensor("qk_new", [4, 128, S], bt, kind="ExternalInput")
    # cs_t: [cosT, sinTpm], each [128=(h d), 1024 tok] fp32 (constant)
    cs_t = nc.dram_tensor("cs_t", [2, 128, S], bt, kind="ExternalInput")
    out = nc.dram_tensor("out", [2, 2, D, 512], dt, kind="ExternalOutput")

    with tile.TileContext(nc) as tc:
        with tc.tile_pool(name="const", bufs=1) as constp, \
             tc.tile_pool(name="pers", bufs=1) as pers, \
             tc.tile_pool(name="wk", bufs=4) as wkp, \
             tc.tile_pool(name="ptp", bufs=12) as ptp, \
             tc.tile_pool(name="stp", bufs=DEPTH, space="PSUM") as stp, \
             tc.tile_pool(name="pvp", bufs=1, space="PSUM") as pvp, \
             tc.tile_pool(name="osbp", bufs=1) as osbp, \
             tc.tile_pool(name="finp", bufs=4) as finp:

            # ---- hoisted constants (written once, read by every body) ----
            dume = constp.tile([128, 2], dt, name="dume", tag="dume")
            nc.vector.memset(dume[:, :], 0.0)
            nc.scalar.activation(dume[:, :], dume[:, :], AF.Exp)
            # cs_sb rides the gpsimd SWDGE queue so the sync queue's first
            # transfer is the startup-critical qk-q DMA, not this constant.
            cs_sb = constp.tile([128, 2, S], bt, name="cs_sb", tag="cs_sb")
            nc.gpsimd.dma_start(cs_sb[:, :, :],
                                cs_t.ap().rearrange("c p f -> p c f"))

            # ---- per-phase persistent tiles ----
            ph = []
            for p in range(2):
                d = {
                    "KT": pers.tile([128, W], bt, name=f"KT{p}",
                                    tag=f"KT{p}"),
                    "QT": pers.tile([128, S], bt, name=f"QT{p}",
                                    tag=f"QT{p}"),
                    # V weight cols padded 65->128: FWL (4x faster
                    # LDWEIGHTS) needs exactly 128 non-fp32 columns; the
                    # pad columns hold garbage and only feed PSUM
                    # partitions 65-127, which are never read.
                    "V0": pers.tile([128, NKB, 128], bt, name=f"V0_{p}",
                                    tag=f"V0_{p}"),
                    "V1": pers.tile([128, NKB, 128], bt, name=f"V1_{p}",
                                    tag=f"V1_{p}"),
                    "qk": pers.tile([128, 4, S], bt, name=f"qk{p}",
                                    tag=f"qk{p}"),
                }
                ph.append(d)

            env = {
                "nc": nc, "ds": ds, "dt": dt, "bt": bt, "AF": AF,
                "ALU": ALU, "kt_old": kt_old, "v_all": v_all,
                "qk_new": qk_new, "out": out,
                "wkp": wkp, "ptp": ptp, "stp": stp, "pvp": pvp,
                "osbp": osbp, "finp": finp, "cs_sb": cs_sb,
                "ph": ph, "handoff": {},
            }

            if loop:
                assert niters % UNROLL == 0, (niters, UNROLL)
                for op in _emit_load(env, 0):
                    op()
                with tc.For_i(0, niters // UNROLL, 1) as _i:
                    for u in range(UNROLL):
                        cur, nxt = u % 2, (u + 1) % 2
                        for op in _emit_load(env, nxt):
                            op()
                        _emit_attn(env, cur,
                                   preemit_next=(u + 1 < UNROLL))
            else:
                for it in range(niters):
                    cur = it % 2
                    if it == 0:
                        for op in _emit_load(env, 0):
                            op()
                    if it + 1 < niters:
                        for op in _emit_load(env, (it + 1) % 2):
                            op()
                        _emit_attn(env, cur, preemit_next=True)
                    else:
                        _emit_attn(env, cur, preemit_next=False)

    nc.compile()
    return nc


def _emit_load(env, p):
    """DMAs + rope for phase p. Rope is three DVE tensor ops per tensor
    (host supplies the transposed + pair-swapped operands and the
    sign-baked trig table), writing QT / KT[:, 7168:] directly in bf16."""
    nc, ds, dt, bt = env["nc"], env["ds"], env["dt"], env["bt"]
    kt_old, v_all, qk_new = env["kt_old"], env["v_all"], env["qk_new"]
    wkp, cs_sb = env["wkp"], env["cs_sb"]
    t = env["ph"][p]
    KT, Vt, qk = t["KT"], [t["V0"], t["V1"]], t["qk"]

    # Critical startup path: q-pair DMA -> rope-q -> first QK batches. Ship
    # the q halves of qk_new and the first KT chunk before the k halves
    # (rope-k only feeds KT[7168:], consumed by the last kb batches).
    nc.sync.dma_start(qk[:, 0:2, :],
                      qk_new.ap()[0:2].rearrange("c p f -> p c f"))
    nc.sync.dma_start(KT[:, ds(0, 1024)], kt_old.ap()[:, ds(0, 1024)])
    nc.sync.dma_start(qk[:, 2:4, :],
                      qk_new.ap()[2:4].rearrange("c p f -> p c f"))

    def vchunk(h, j):
        nc.gpsimd.dma_start(Vt[h][:, ds(j * 16, 16), :],
                            v_all.ap()[h][:, ds(j * 16, 16), :])

    vchunk(0, 0)
    vchunk(1, 0)
    vorder = [(0, 1), (1, 1), (0, 2), (1, 2), (0, 3), (1, 3)]
    for i in range(1, 7):
        nc.sync.dma_start(KT[:, ds(i * 1024, 1024)],
                          kt_old.ap()[:, ds(i * 1024, 1024)])
        if vorder:
            vchunk(*vorder.pop(0))
    while vorder:
        vchunk(*vorder.pop(0))

    # rope: out = xT*cosT + xTswap*sinTpm, as closures (callers run them
    # immediately). rope-q is split into column halves so QT[:, 0:512]
    # (all the qt0 QK batches need) is ready one half-rope earlier on the
    # single-shot critical path.
    cosT, sinT = cs_sb[:, 0, :], cs_sb[:, 1, :]
    ops = []
    pieces = [("q", t["QT"], 0, ds(0, 512)), ("q", t["QT"], 0, ds(512, 512)),
              ("k", None, 2, ds(0, S))]
    for which, qdst, base, sl in pieces:
        dst = qdst[:, sl] if which == "q" else KT[:, ds(OLD, S)]
        n = sl.size if hasattr(sl, "size") else S
        ta = wkp.tile([128, S], bt, tag="rt", bufs=4,
                      name=f"r{which}a{p}{sl.start}")
        tb = wkp.tile([128, S], bt, tag="rt", bufs=4,
                      name=f"r{which}b{p}{sl.start}")
        ops.append(lambda ta=ta, base=base, sl=sl: nc.vector.tensor_mul(
            ta[:, sl], qk[:, base, sl], cosT[:, sl]))
        ops.append(lambda tb=tb, base=base, sl=sl: nc.vector.tensor_mul(
            tb[:, sl], qk[:, base + 1, sl], sinT[:, sl]))
        ops.append(lambda dst=dst, ta=ta, tb=tb, sl=sl: nc.vector.tensor_add(
            dst, ta[:, sl], tb[:, sl]))
    return ops


def _emit_attn(env, p, preemit_next=False, inject=()):
    """QK -> exp -> PV over one merged 256-slice stream for phase p.

    `inject` is a list of deferred DVE closures (next body's rope ops),
    spread one per ~12 batches so the DVE FIFO never backs up."""
    nc, ds, dt, bt = env["nc"], env["ds"], env["dt"], env["bt"]
    AF, ALU, out = env["AF"], env["ALU"], env["out"]
    ptp, stp, pvp = env["ptp"], env["stp"], env["pvp"]
    osbp, finp = env["osbp"], env["finp"]
    t = env["ph"][p]
    KT, QT, Vt = t["KT"], t["QT"], [t["V0"], t["V1"]]

    osb = {}
    pvts = {}

    def emit_osb(qt, h):
        ot = osbp.tile([65, 512], dt, tag=f"osb{qt}{h}",
                       bufs=1, name=f"osb{p}_{qt}{h}")
        nc.vector.tensor_copy(ot[:], pvts[(qt, h)][0:65, :])
        osb[(qt, h)] = ot

    def emit_tail(qt, h, direct=False):
        # Normalize in the [d, tok] layout (no transpose): reciprocal of
        # the denominator row, partition-broadcast it on the idle GpSimd
        # engine, multiply, DMA out. Host transposes to [tok, d].
        # direct=True (final qt1 tails): read the PV accumulator straight
        # from PSUM — the osb copy only exists to free the bank early for
        # qt1's accumulation, which the last tiles don't need.
        ot = pvts[(qt, h)] if direct else osb[(qt, h)]
        rec = finp.tile([1, 512], dt, tag="rec", bufs=2,
                        name=f"rec{p}_{qt}{h}")
        nc.vector.reciprocal(rec[:, :], ot[64:65, :])
        rb = finp.tile([64, 512], dt, tag="rb", bufs=2,
                       name=f"rb{p}_{qt}{h}")
        nc.gpsimd.partition_broadcast(rb[:, :], rec[:, :])
        fin = finp.tile([64, 512], dt, tag="fin", bufs=2,
                        name=f"fin{p}_{qt}{h}")
        nc.vector.tensor_tensor(fin[:, :], ot[0:64, :], rb[:, :], ALU.mult)
        nc.sync.dma_start(out.ap()[qt, h], fin[:, :])

    def get_pvt(qt, h):
        if (qt, h) not in pvts:
            pvts[(qt, h)] = pvp.tile([128, 512], dt, tag=f"pv{h}", bufs=1,
                                     name=f"pv{p}_{qt}{h}")
        return pvts[(qt, h)]

    # Slice = (qt, kb, h); one merged stream over both q-tiles so the qt
    # transition costs no ScalarE gap.
    slices = [(qt, kb, h)
              for qt in range(2) for kb in range(NKB) for h in range(2)]
    batches = [slices[b0:b0 + BATCH] for b0 in range(0, len(slices), BATCH)]
    sts = env["handoff"]

    def emit_qk(b, tiles, phase):
        KTx, QTx = tiles
        batch = batches[b]
        st = stp.tile([128, BATCH, 512], dt, tag="st", bufs=DEPTH,
                      name=f"st{phase}_{b}")
        for i, (qt, kb, h) in enumerate(batch):
            nc.tensor.matmul(
                st[:, i, :],
                lhsT=KTx[64 * h:64 * h + 64, ds(kb * 128, 128)],
                rhs=QTx[64 * h:64 * h + 64, ds(qt * 512, 512)],
                start=True, stop=True,
                tile_position=(64 * h, 0),
            )
        sts[b] = st

    for j in range(DEPTH):
        if j not in sts:
            emit_qk(j, (KT, QT), p)
    import concourse.mybir as mybir
    it = mybir.dt.int16
    inject = list(inject)
    pts = {}

    def emit_pv(b):
        pt = pts.pop(b)
        for i, (qt, kb, h) in enumerate(batches[b]):
            if qt == 1 and kb == 0:
                # qt1 reuses qt0's PSUM accumulator bank: copy qt0's
                # result to SBUF first (the WAR on that copy orders it).
                emit_osb(0, h)
                emit_tail(0, h)
            nc.tensor.matmul(
                get_pvt(qt, h),
                lhsT=Vt[h][:, kb, :],
                rhs=pt[:, i, :],
                start=(kb == 0), stop=(kb == NKB - 1),
            )

    for b, batch in enumerate(batches):
        if inject and b >= 8 and (b - 8) % 20 == 0:
            inject.pop(0)()
        nb = len(batch)
        st = sts.pop(b)
        pt = ptp.tile([128, BATCH, 512], bt, tag="pt", bufs=12,
                      name=f"pt{p}_{b}")
        pts[b] = pt
        # Split the batch's slices between ScalarE (true exp) and the DVE
        # (fast-exp: int16 bits written through a bf16 bitcast view of pt).
        nv = _carve(b, nb)
        nsc = nb - nv
        if nsc > 0:
            nc.scalar.activation(pt[:, 0:nsc, :], st[:, 0:nsc, :],
                                 AF.Exp, scale=SCALE)
        if nv > 0:
            nc.vector.tensor_scalar(pt[:, nsc:nb, :].bitcast(it),
                                    st[:, nsc:nb, :], FA, FB,
                                    ALU.mult, ALU.add)
        # QK(b+DEPTH) rides behind exp/carve(b); the DEPTH-deep st
        # rotation gives the bank-recycle chain ~2 batch-periods of
        # slack, so DVE FIFO jitter never stalls the PE or ScalarE.
        if b + DEPTH < len(batches):
            emit_qk(b + DEPTH, (KT, QT), p)
        elif preemit_next:
            # Pre-emit the next body's first QK batches in the slots
            # freed by the last exps, ahead of the trailing PVs, so the
            # next body's exps start with no PE work on the critical
            # path at the boundary.
            np_ = 1 - p
            tn = env["ph"][np_]
            emit_qk(b + DEPTH - len(batches), (tn["KT"], tn["QT"]), np_)
        # PV trails by PVD batches so the PE never reaches a PV whose pt
        # isn't ready yet (head-of-line blocking in the PE FIFO).
        if b >= PVD:
            emit_pv(b - PVD)
    for b in range(len(batches) - PVD, len(batches)):
        emit_pv(b)
    for h in range(2):
        if preemit_next:
            # Mid-loop: copy out via SBUF so the pv bank frees before the
            # next body's qt0 accumulation claims it.
            emit_osb(1, h)
            emit_tail(1, h)
        else:
            emit_tail(1, h, direct=True)


def _prep_inputs(q, k, v, cache_k, cache_v, freqs_cos, freqs_sin):
    """Host-side sharding + layout prep (no FLOPs beyond data movement)."""
    import ml_dtypes
    bf16 = ml_dtypes.bfloat16
    q = np.asarray(q, np.float32)
    k = np.asarray(k, np.float32)
    v = np.asarray(v, np.float32)
    cache_k = np.asarray(cache_k, np.float32)
    cache_v = np.asarray(cache_v, np.float32)
    cos_h = np.asarray(freqs_cos, np.float32)[START:START + S, 0::2]
    sin_h = np.asarray(freqs_sin, np.float32)[START:START + S, 0::2]

    # cosT/sinTpm [128=(h d), tok]: row (h, d) carries cos/sin[:, d//2];
    # sin rows get the rope sign pattern (-1 on even d, +1 on odd d).
    cosT = np.tile(np.repeat(cos_h.T, 2, axis=0), (2, 1))      # [128, 1024]
    sg = np.tile(np.array([-1.0, 1.0], np.float32), 64)[:, None]
    sinT = np.tile(np.repeat(sin_h.T, 2, axis=0), (2, 1)) * sg
    cs_t = np.ascontiguousarray(np.stack([cosT, sinT])).astype(bf16)

    def tpose_pair(x):  # x [tok, 128] -> (xT, xT with d-pairs swapped)
        xT = np.ascontiguousarray(x.T)                         # [128, tok]
        xTs = np.ascontiguousarray(
            xT.reshape(64, 2, S)[:, ::-1, :].reshape(128, S))
        return xT, xTs

    in_maps = []
    for c in range(NCORES):
        hs = slice(2 * c, 2 * c + 2)
        k_old = cache_k[0, CLO:CHI, hs, :]                      # [7168, 2, 64]
        kt_old = np.ascontiguousarray(
            k_old.transpose(1, 2, 0).reshape(128, OLD)).astype(bf16)
        # V window (old cache rows + raw new v), laid out [h, p, kb, 128
        # weight cols]: cols 0:64 = V, col 64 = 1.0 (denominator), rest 0.
        vw = np.concatenate([cache_v[0, CLO:CHI, hs, :],
                             v[0, :, hs, :]], axis=0)           # [8192, 2, 64]
        v_all = np.zeros((2, 128, NKB, 128), np.float32)
        v_all[:, :, :, 0:64] = vw.reshape(NKB, 128, 2, D).transpose(2, 1, 0, 3)
        v_all[:, :, :, 64] = 1.0
        v_all = np.ascontiguousarray(v_all).astype(bf16)
        qT, qTs = tpose_pair(q[0, :, hs, :].reshape(S, 128))
        kT, kTs = tpose_pair(k[0, :, hs, :].reshape(S, 128))
        qk_new = np.ascontiguousarray(np.stack([qT, qTs, kT, kTs])).astype(bf16)
        in_maps.append({
            "kt_old": kt_old, "v_all": v_all, "qk_new": qk_new,
            "cs_t": cs_t,
        })
    return in_maps


def get_nc(niters=1, loop=False):
    key = ("nc", niters, loop)
    if key not in _cache:
        _cache[key] = _build(niters, loop)
    return _cache[key]


def _make_runner(nc, n_cores=NCORES):
    """Reusable jitted SPMD callable (mirrors bass2jax.run_bass_via_pjrt)
    so repeat kernel() calls skip retracing/compilation."""
    import jax
    from jax.experimental.shard_map import shard_map
    from jax.sharding import Mesh, NamedSharding, PartitionSpec

    import concourse.mybir as mybir
    from concourse.bass2jax import (_bass_exec_p, install_neuronx_cc_hook,
                                    partition_id_tensor)

    install_neuronx_cc_hook()
    partition_name = (nc.partition_id_tensor.name
                      if nc.partition_id_tensor else None)
    in_names, out_names, out_avals, zero_outs = [], [], [], []
    for alloc in nc.m.functions[0].allocations:
        if not isinstance(alloc, mybir.MemoryLocationSet):
            continue
        name = alloc.memorylocations[0].name
        if alloc.kind == "ExternalInput":
            if name != partition_name:
                in_names.append(name)
        elif alloc.kind == "ExternalOutput":
            shape = tuple(alloc.tensor_shape)
            dtype = mybir.dt.np(alloc.dtype)
            out_names.append(name)
            out_avals.append(jax.core.ShapedArray(shape, dtype))
            zero_outs.append(np.zeros(shape, dtype))
    n_params = len(in_names)
    n_outs = len(out_avals)
    all_in_names = list(in_names) + out_names
    if partition_name is not None:
        all_in_names.append(partition_name)

    def _body(*args):
        operands = list(args)
        if partition_name is not None:
            operands.append(partition_id_tensor())
        return tuple(_bass_exec_p.bind(
            *operands,
            out_avals=tuple(out_avals),
            in_names=tuple(all_in_names),
            out_names=tuple(out_names),
            lowering_input_output_aliases=(),
            sim_require_finite=True,
            sim_require_nnan=True,
            nc=nc,
        ))

    devices = jax.devices()[:n_cores]
    mesh = Mesh(np.asarray(devices), ("core",))
    sharded = jax.jit(
        shard_map(_body, mesh=mesh,
                  in_specs=(PartitionSpec("core"),) * (n_params + n_outs),
                  out_specs=(PartitionSpec("core"),) * n_outs,
                  check_rep=False),
        donate_argnums=tuple(range(n_params, n_params + n_outs)),
        keep_unused=True)
    sharding = NamedSharding(mesh, PartitionSpec("core"))

    def call(in_maps):
        concat_in = [
            np.concatenate([np.asarray(in_maps[c][nm])
                            for c in range(n_cores)], axis=0)
            for nm in in_names]
        zs = [np.zeros((n_cores * z.shape[0], *z.shape[1:]), z.dtype)
              for z in zero_outs]
        args = [jax.device_put(a, sharding) for a in concat_in + zs]
        outs = sharded(*args)
        return [
            {nm: np.asarray(outs[i]).reshape(n_cores, *out_avals[i].shape)[c]
             for i, nm in enumerate(out_names)}
            for c in range(n_cores)]

    return call


def _run(in_maps, niters=1):
    from concourse.bass_utils import run_bass_kernel_spmd
    res = run_bass_kernel_spmd(get_nc(niters), in_maps,
                               core_ids=list(range(NCORES)))
    out_full = np.empty((1, S, H, D), np.float32)
    for c in range(NCORES):
        _scatter_out(out_full, res.results[c]["out"], c)
    return out_full.reshape(1, S, H * D), res


def _scatter_out(out_full, arr, c):
    # device layout [qt, h, d, tok] -> tokens qt*512 + tok
    arr = np.asarray(arr).reshape(2, 2, D, 512)
    for qt in range(2):
        for h in range(2):
            out_full[0, qt * 512:(qt + 1) * 512, 2 * c + h, :] = arr[qt, h].T


def kernel(q, k, v, cache_k, cache_v, freqs_cos, freqs_sin):
    in_maps = _prep_inputs(q, k, v, cache_k, cache_v, freqs_cos, freqs_sin)
    try:
        if "runner" not in _cache:
            _cache["runner"] = _make_runner(get_nc(1))
        results = _cache["runner"](in_maps)
    except Exception:
        out, _ = _run(in_maps)
        return out
    out_full = np.empty((1, S, H, D), np.float32)
    for c in range(NCORES):
        _scatter_out(out_full, results[c]["out"], c)
    return out_full.reshape(1, S, H * D)



# revision 28
# speedup vs baseline: 1.0389x; 1.0389x over previous
"""Windowed attention w/ ring-buffer KV cache for TRN2, 8 NeuronCores.

Problem (hardcoded): B=1, S=1024 new tokens, H=16 heads, D=64,
cache C=10240, window W=8192, START_FRAME=9728.

Math (derived from the reference ring-buffer update; the updated cache is
not returned, only the attention output):
  wk = concat(cache_k[2560:9728], rope(k)),  wv = concat(cache_v[2560:9728], v)
  out = softmax(rope(q) @ wk^T / 8) @ wv     (non-causal, all 8192 keys)

Sharding: head-parallel, 2 heads per core (core c owns heads 2c, 2c+1).
Each core computes its full [1024, 2, 64] output slice; host concatenates.

ScalarE exp (16.8M exps/core at 1 elem/cycle/lane ~ 110us floor) and the
PE matmul stream (~512 MMs x ~244ns measured in situ ~ 125us) are jointly
the bottleneck; the kernel keeps both saturated:
  - KT  [128, 8192] bf16: K^T, partitions = (head(2) x d(64)), cols = pos
  - QT  [128, 1024] bf16: Q^T, same partition layout
  - Vp  [128, 64kb, 128] bf16 per head: V natural layout; col 64 = 1.0
    (softmax denominator via the PV matmul) and cols 65:127 = 0 pad are
    BAKED HOST-SIDE into v_all, so no on-device memsets/ones-copies ever
    block the startup path, and the 128 non-fp32 weight cols keep FWL
    (fast weight load) on for PV LDWEIGHTS
  - rope: host ships q/k pre-transposed as [hd, tok] plus a pair-swapped
    copy and a sign-baked cos/sin table (all pure data movement, bf16),
    so rope(x)^T = xT*cosT + xTswap*sinTpm -- three DVE ops straight
    into QT/KT, no PE transposes, no PSUM traffic. The 6 DVE rope ops of
    the NEXT body are returned as closures and injected one-at-a-time
    between attention batches so the DVE FIFO never backs up.
  - QK  : S^T[k,q] via row-tiled bf16 matmuls (head0 rows 0-63, head1
    64-127); adjacent opposite-band MMs run CONCURRENTLY on the PE
    (HW-measured 128 ns/MM alternating vs 473 same-band)
  - exp : ScalarE over 3-bank PSUM batches (scale=1/8 fused), bf16 out;
    QK(b+DEPTH) is emitted after exp(b) so the bank-recycle chain
    exp(b) -> QK(b+DEPTH) -> exp(b+DEPTH) stays off the critical path
  - PV  : accumulate V'^T @ P^T into PSUM [128, 512] per (head, q-tile);
    row 64 = softmax denominator (from the ones column), rows 65-127
    are pad output and never read
  - tail: normalize in [d, tok] layout (reciprocal on DVE, denominator
    partition-broadcast on the idle GpSimd engine, multiply, DMA); the
    host transposes the output back to [tok, d]
  - startup (single-shot): the q halves of qk_new and KT chunk 0 are
    DMA'd before the k halves, so rope-q -> first QK batches begin ~11us
    in instead of ~22us.

A DVE fast-exp offload (Schraudolph int16-bits-as-bf16, FA/FB/_carve
below) is implemented and numerically validated (rel err ~2% rms on
carved slices, end-to-end ~7e-3 vs the 2e-2 gate) but DISABLED
(DVE_PAT empty): on HW, every carved variant measured slower
(127-150us/iter vs 124.7) because the PE matmul stream with per-MM sem
overhead is itself at ~125us/body, so offloading ScalarE work only adds
cross-engine coupling without lowering the wall. It becomes profitable
only if PV moves to fp8 DoubleRow (halving PV PE time) -- untested.

Steady-state pipelining (loop mode): input tiles are double-buffered by
body parity; each body emits the next body's DMAs + rope up front so they
stream during this body's attention, and pre-emits the next body's first
QK batches so ScalarE never idles at the body boundary.
"""

import numpy as np

H, D = 16, 64
S = 1024
W = 8192
OLD = 7168          # window rows taken from old cache (cache rows 2560:9728)
CLO, CHI = 2560, 9728
START = 9728
NCORES = 8
NKB = W // 128      # 64
NQT = S // 128      # 8 token blocks
SCALE = 0.125
UNROLL = 20         # bodies per For_i iteration (amortizes the barrier)

# DVE fast-exp (Schraudolph in bf16 bit-space): for a carved slice the DVE
# computes int16(rint(x*FA + FB)) whose bits, read as bf16, are ~exp(x/8)
# (rel err ~2% rms, zero-mean; softmax averaging washes it out). One DVE
# tensor_scalar per carved slice vs one ScalarE exp element-slot — this
# offloads ~35% of the exp work to the otherwise-idle Vector engine.
FA = 128.0 * float(np.log2(np.e)) / 8.0   # folds the 1/8 score scale
FB = 127.0 * 128.0 - 7.25                 # exponent bias + sawtooth centering
BATCH = 2           # slices per st tile (2 PSUM banks)
DEPTH = 3           # st rotation depth: QK(b+DEPTH) overlaps exp/carve(b)
# Whole-batch carve pattern: a batch is processed entirely by ScalarE (true
# exp) or entirely by the DVE (fast-exp). Whole batches mean one instruction
# per batch per engine and halve each engine's bank-recycle cadence, hiding
# cross-engine semaphore latency. 5/12 of batches -> DVE balances
# ScalarE ~77us vs DVE ~75us per body.
DVE_PAT = frozenset((1, 3, 5, 8, 10))
DVE_MOD = 12
PVD = 0             # PV trails the exp stream by this many batches

_cache = {}


def _carve(b, nb):
    """Slices of batch b the DVE takes: all of them or none."""
    return nb if b % DVE_MOD in DVE_PAT else 0


def _build(niters=1, loop=False):
    import concourse.mybir as mybir
    import concourse.tile as tile
    from concourse import bacc
    from concourse._compat import axon_active
    from concourse.bass import ds

    dt = mybir.dt.float32
    bt = mybir.dt.bfloat16
    AF = mybir.ActivationFunctionType
    ALU = mybir.AluOpType

    nc = bacc.Bacc(
        "TRN2", target_bir_lowering=False, debug=not axon_active(),
        num_devices=NCORES,
    )
    kt_old = nc.dram_tensor("kt_old", [128, OLD], bt, kind="ExternalInput")
    # v_all carries the full 128 weight cols per head: cols 0:64 = V, col 64
    # = 1.0 (softmax denominator via the PV matmul), 65:128 = 0 pad (FWL
    # needs 128 non-fp32 weight cols). Baked host-side so no on-device
    # memsets/ones-copies are needed.
    v_all = nc.dram_tensor("v_all", [2, 128, NKB, 128], bt, kind="ExternalInput")
    # qk_new: [qT, qTswap, kT, kTswap], each [128=(h d), 1024 tok] fp32
    qk_new = nc.dram_tensor("qk_new", [4, 128, S], bt, kind="ExternalInput")
    # cs_t: [cosT, sinTpm], each [128=(h d), 1024 tok] fp32 (constant)
    cs_t = nc.dram_tensor("cs_t", [2, 128, S], bt, kind="ExternalInput")
    out = nc.dram_tensor("out", [2, 2, D, 512], dt, kind="ExternalOutput")

    with tile.TileContext(nc) as tc:
        with tc.tile_pool(name="const", bufs=1) as constp, \
             tc.tile_pool(name="pers", bufs=1) as pers, \
             tc.tile_pool(name="wk", bufs=4) as wkp, \
             tc.tile_pool(name="ptp", bufs=12) as ptp, \
             tc.tile_pool(name="stp", bufs=DEPTH, space="PSUM") as stp, \
             tc.tile_pool(name="pvp", bufs=1, space="PSUM") as pvp, \
             tc.tile_pool(name="osbp", bufs=1) as osbp, \
             tc.tile_pool(name="finp", bufs=4) as finp:

            # ---- hoisted constants (written once, read by every body) ----
            dume = constp.tile([128, 2], dt, name="dume", tag="dume")
            nc.vector.memset(dume[:, :], 0.0)
            nc.scalar.activation(dume[:, :], dume[:, :], AF.Exp)
            # cs_sb rides the gpsimd SWDGE queue so the sync queue's first
            # transfer is the startup-critical qk-q DMA, not this constant.
            cs_sb = constp.tile([128, 2, S], bt, name="cs_sb", tag="cs_sb")
            nc.gpsimd.dma_start(cs_sb[:, :, :],
                                cs_t.ap().rearrange("c p f -> p c f"))

            # ---- per-phase persistent tiles ----
            ph = []
            for p in range(2):
                d = {
                    "KT": pers.tile([128, W], bt, name=f"KT{p}",
                                    tag=f"KT{p}"),
                    "QT": pers.tile([128, S], bt, name=f"QT{p}",
                                    tag=f"QT{p}"),
                    # V weight cols padded 65->128: FWL (4x faster
                    # LDWEIGHTS) needs exactly 128 non-fp32 columns; the
                    # pad columns hold garbage and only feed PSUM
                    # partitions 65-127, which are never read.
                    "V0": pers.tile([128, NKB, 128], bt, name=f"V0_{p}",
                                    tag=f"V0_{p}"),
                    "V1": pers.tile([128, NKB, 128], bt, name=f"V1_{p}",
                                    tag=f"V1_{p}"),
                    "qk": pers.tile([128, 4, S], bt, name=f"qk{p}",
                                    tag=f"qk{p}"),
                }
                ph.append(d)

            env = {
                "nc": nc, "ds": ds, "dt": dt, "bt": bt, "AF": AF,
                "ALU": ALU, "kt_old": kt_old, "v_all": v_all,
                "qk_new": qk_new, "out": out,
                "wkp": wkp, "ptp": ptp, "stp": stp, "pvp": pvp,
                "osbp": osbp, "finp": finp, "cs_sb": cs_sb,
                "ph": ph, "handoff": {},
            }

            if loop:
                assert niters % UNROLL == 0, (niters, UNROLL)
                for op in _emit_load(env, 0):
                    op()
                with tc.For_i(0, niters // UNROLL, 1) as _i:
                    for u in range(UNROLL):
                        cur, nxt = u % 2, (u + 1) % 2
                        for op in _emit_load(env, nxt):
                            op()
                        _emit_attn(env, cur,
                                   preemit_next=(u + 1 < UNROLL))
            else:
                for it in range(niters):
                    cur = it % 2
                    if it == 0:
                        for op in _emit_load(env, 0):
                            op()
                    if it + 1 < niters:
                        for op in _emit_load(env, (it + 1) % 2):
                            op()
                        _emit_attn(env, cur, preemit_next=True)
                    else:
                        _emit_attn(env, cur, preemit_next=False)

    nc.compile()
    return nc


def _emit_load(env, p):
    """DMAs + rope for phase p. Rope is three DVE tensor ops per tensor
    (host supplies the transposed + pair-swapped operands and the
    sign-baked trig table), writing QT / KT[:, 7168:] directly in bf16."""
    nc, ds, dt, bt = env["nc"], env["ds"], env["dt"], env["bt"]
    kt_old, v_all, qk_new = env["kt_old"], env["v_all"], env["qk_new"]
    wkp, cs_sb = env["wkp"], env["cs_sb"]
    t = env["ph"][p]
    KT, Vt, qk = t["KT"], [t["V0"], t["V1"]], t["qk"]

    # Critical startup path: q-pair DMA -> rope-q -> first QK batches. Ship
    # the q halves of qk_new and the first KT chunk before the k halves
    # (rope-k only feeds KT[7168:], consumed by the last kb batches).
    nc.sync.dma_start(qk[:, 0:2, :],
                      qk_new.ap()[0:2].rearrange("c p f -> p c f"))
    nc.sync.dma_start(KT[:, ds(0, 1024)], kt_old.ap()[:, ds(0, 1024)])
    nc.sync.dma_start(qk[:, 2:4, :],
                      qk_new.ap()[2:4].rearrange("c p f -> p c f"))

    def vchunk(h, j):
        nc.gpsimd.dma_start(Vt[h][:, ds(j * 16, 16), :],
                            v_all.ap()[h][:, ds(j * 16, 16), :])

    vchunk(0, 0)
    vchunk(1, 0)
    vorder = [(0, 1), (1, 1), (0, 2), (1, 2), (0, 3), (1, 3)]
    for i in range(1, 7):
        nc.sync.dma_start(KT[:, ds(i * 1024, 1024)],
                          kt_old.ap()[:, ds(i * 1024, 1024)])
        if vorder:
            vchunk(*vorder.pop(0))
    while vorder:
        vchunk(*vorder.pop(0))

    # rope: out = xT*cosT + xTswap*sinTpm, as closures (callers run them
    # immediately). rope-q is split into column halves so QT[:, 0:512]
    # (all the qt0 QK batches need) is ready one half-rope earlier on the
    # single-shot critical path.
    cosT, sinT = cs_sb[:, 0, :], cs_sb[:, 1, :]
    ops = []
    pieces = [("q", t["QT"], 0, ds(0, 512)), ("q", t["QT"], 0, ds(512, 512)),
              ("k", None, 2, ds(0, S))]
    for which, qdst, base, sl in pieces:
        dst = qdst[:, sl] if which == "q" else KT[:, ds(OLD, S)]
        n = sl.size if hasattr(sl, "size") else S
        ta = wkp.tile([128, S], bt, tag="rt", bufs=4,
                      name=f"r{which}a{p}{sl.start}")
        tb = wkp.tile([128, S], bt, tag="rt", bufs=4,
                      name=f"r{which}b{p}{sl.start}")
        ops.append(lambda ta=ta, base=base, sl=sl: nc.vector.tensor_mul(
            ta[:, sl], qk[:, base, sl], cosT[:, sl]))
        ops.append(lambda tb=tb, base=base, sl=sl: nc.vector.tensor_mul(
            tb[:, sl], qk[:, base + 1, sl], sinT[:, sl]))
        ops.append(lambda dst=dst, ta=ta, tb=tb, sl=sl: nc.vector.tensor_add(
            dst, ta[:, sl], tb[:, sl]))
    return ops


def _emit_attn(env, p, preemit_next=False, inject=()):
    """QK -> exp -> PV over one merged 256-slice stream for phase p.

    `inject` is a list of deferred DVE closures (next body's rope ops),
    spread one per ~12 batches so the DVE FIFO never backs up."""
    nc, ds, dt, bt = env["nc"], env["ds"], env["dt"], env["bt"]
    AF, ALU, out = env["AF"], env["ALU"], env["out"]
    ptp, stp, pvp = env["ptp"], env["stp"], env["pvp"]
    osbp, finp = env["osbp"], env["finp"]
    t = env["ph"][p]
    KT, QT, Vt = t["KT"], t["QT"], [t["V0"], t["V1"]]

    osb = {}
    pvts = {}

    def emit_osb(qt, h):
        ot = osbp.tile([65, 512], dt, tag=f"osb{qt}{h}",
                       bufs=1, name=f"osb{p}_{qt}{h}")
        nc.vector.tensor_copy(ot[:], pvts[(qt, h)][0:65, :])
        osb[(qt, h)] = ot

    def emit_tail(qt, h, direct=False):
        # Normalize in the [d, tok] layout (no transpose): reciprocal of
        # the denominator row, partition-broadcast it on the idle GpSimd
        # engine, multiply, DMA out. Host transposes to [tok, d].
        # direct=True (final qt1 tails): read the PV accumulator straight
        # from PSUM — the osb copy only exists to free the bank early for
        # qt1's accumulation, which the last tiles don't need.
        ot = pvts[(qt, h)] if direct else osb[(qt, h)]
        rec = finp.tile([1, 512], dt, tag="rec", bufs=2,
                        name=f"rec{p}_{qt}{h}")
        nc.vector.reciprocal(rec[:, :], ot[64:65, :])
        rb = finp.tile([64, 512], dt, tag="rb", bufs=2,
                       name=f"rb{p}_{qt}{h}")
        nc.gpsimd.partition_broadcast(rb[:, :], rec[:, :])
        fin = finp.tile([64, 512], dt, tag="fin", bufs=2,
                        name=f"fin{p}_{qt}{h}")
        nc.vector.tensor_tensor(fin[:, :], ot[0:64, :], rb[:, :], ALU.mult)
        nc.sync.dma_start(out.ap()[qt, h], fin[:, :])

    def get_pvt(qt, h):
        if (qt, h) not in pvts:
            pvts[(qt, h)] = pvp.tile([128, 512], dt, tag=f"pv{h}", bufs=1,
                                     name=f"pv{p}_{qt}{h}")
        return pvts[(qt, h)]

    # Slice = (qt, kb, h); one merged stream over both q-tiles so the qt
    # transition costs no ScalarE gap.
    slices = [(qt, kb, h)
              for qt in range(2) for kb in range(NKB) for h in range(2)]
    batches = [slices[b0:b0 + BATCH] for b0 in range(0, len(slices), BATCH)]
    sts = env["handoff"]

    def emit_qk(b, tiles, phase):
        KTx, QTx = tiles
        batch = batches[b]
        st = stp.tile([128, BATCH, 512], dt, tag="st", bufs=DEPTH,
                      name=f"st{phase}_{b}")
        for i, (qt, kb, h) in enumerate(batch):
            nc.tensor.matmul(
                st[:, i, :],
                lhsT=KTx[64 * h:64 * h + 64, ds(kb * 128, 128)],
                rhs=QTx[64 * h:64 * h + 64, ds(qt * 512, 512)],
                start=True, stop=True,
                tile_position=(64 * h, 0),
            )
        sts[b] = st

    for j in range(DEPTH):
        if j not in sts:
            emit_qk(j, (KT, QT), p)
    import concourse.mybir as mybir
    it = mybir.dt.int16
    inject = list(inject)
    pts = {}

    def emit_pv(b):
        pt = pts.pop(b)
        for i, (qt, kb, h) in enumerate(batches[b]):
            if qt == 1 and kb == 0:
                # qt1 reuses qt0's PSUM accumulator bank: copy qt0's
                # result to SBUF first (the WAR on that copy orders it).
                emit_osb(0, h)
                emit_tail(0, h)
            nc.tensor.matmul(
                get_pvt(qt, h),
                lhsT=Vt[h][:, kb, :],
                rhs=pt[:, i, :],
                start=(kb == 0), stop=(kb == NKB - 1),
            )

    for b, batch in enumerate(batches):
        if inject and b >= 8 and (b - 8) % 20 == 0:
            inject.pop(0)()
        nb = len(batch)
        st = sts.pop(b)
        pt = ptp.tile([128, BATCH, 512], bt, tag="pt", bufs=12,
                      name=f"pt{p}_{b}")
        pts[b] = pt
        # Split the batch's slices between ScalarE (true exp) and the DVE
        # (fast-exp: int16 bits written through a bf16 bitcast view of pt).
        nv = _carve(b, nb)
        nsc = nb - nv
        if nsc > 0:
            nc.scalar.activation(pt[:, 0:nsc, :], st[:, 0:nsc, :],
                                 AF.Exp, scale=SCALE)
        if nv > 0:
            nc.vector.tensor_scalar(pt[:, nsc:nb, :].bitcast(it),
                                    st[:, nsc:nb, :], FA, FB,
                                    ALU.mult, ALU.add)
        # QK(b+DEPTH) rides behind exp/carve(b); the DEPTH-deep st
        # rotation gives the bank-recycle chain ~2 batch-periods of
        # slack, so DVE FIFO jitter never stalls the PE or ScalarE.
        if b + DEPTH < len(batches):
            emit_qk(b + DEPTH, (KT, QT), p)
        elif preemit_next:
            # Pre-emit the next body's first QK batches in the slots
            # freed by the last exps, ahead of the trailing PVs, so the
            # next body's exps start with no PE work on the critical
            # path at the boundary.
            np_ = 1 - p
            tn = env["ph"][np_]
            emit_qk(b + DEPTH - len(batches), (tn["KT"], tn["QT"]), np_)
        # PV trails by PVD batches so the PE never reaches a PV whose pt
        # isn't ready yet (head-of-line blocking in the PE FIFO).
        if b >= PVD:
            emit_pv(b - PVD)
    for b in range(len(batches) - PVD, len(batches)):
        emit_pv(b)
    for h in range(2):
        if preemit_next:
            # Mid-loop: copy out via SBUF so the pv bank frees before the
            # next body's qt0 accumulation claims it.
            emit_osb(1, h)
            emit_tail(1, h)
        else:
            emit_tail(1, h, direct=True)


def _prep_inputs(q, k, v, cache_k, cache_v, freqs_cos, freqs_sin):
    """Host-side sharding + layout prep (no FLOPs beyond data movement)."""
    import ml_dtypes
    bf16 = ml_dtypes.bfloat16
    q = np.asarray(q, np.float32)
    k = np.asarray(k, np.float32)
    v = np.asarray(v, np.float32)
    cache_k = np.asarray(cache_k, np.float32)
    cache_v = np.asarray(cache_v, np.float32)
    cos_h = np.asarray(freqs_cos, np.float32)[START:START + S, 0::2]
    sin_h = np.asarray(freqs_sin, np.float32)[START:START + S, 0::2]

    # cosT/sinTpm [128=(h d), tok]: row (h, d) carries cos/sin[:, d//2];
    # sin rows get the rope sign pattern (-1 on even d, +1 on odd d).
    cosT = np.tile(np.repeat(cos_h.T, 2, axis=0), (2, 1))      # [128, 1024]
    sg = np.tile(np.array([-1.0, 1.0], np.float32), 64)[:, None]
    sinT = np.tile(np.repeat(sin_h.T, 2, axis=0), (2, 1)) * sg
    cs_t = np.ascontiguousarray(np.stack([cosT, sinT])).astype(bf16)

    def tpose_pair(x):  # x [tok, 128] -> (xT, xT with d-pairs swapped)
        xT = np.ascontiguousarray(x.T)                         # [128, tok]
        xTs = np.ascontiguousarray(
            xT.reshape(64, 2, S)[:, ::-1, :].reshape(128, S))
        return xT, xTs

    in_maps = []
    for c in range(NCORES):
        hs = slice(2 * c, 2 * c + 2)
        k_old = cache_k[0, CLO:CHI, hs, :]                      # [7168, 2, 64]
        kt_old = np.ascontiguousarray(
            k_old.transpose(1, 2, 0).reshape(128, OLD)).astype(bf16)
        # V window (old cache rows + raw new v), laid out [h, p, kb, 128
        # weight cols]: cols 0:64 = V, col 64 = 1.0 (denominator), rest 0.
        vw = np.concatenate([cache_v[0, CLO:CHI, hs, :],
                             v[0, :, hs, :]], axis=0)           # [8192, 2, 64]
        v_all = np.zeros((2, 128, NKB, 128), np.float32)
        v_all[:, :, :, 0:64] = vw.reshape(NKB, 128, 2, D).transpose(2, 1, 0, 3)
        v_all[:, :, :, 64] = 1.0
        v_all = np.ascontiguousarray(v_all).astype(bf16)
        qT, qTs = tpose_pair(q[0, :, hs, :].reshape(S, 128))
        kT, kTs = tpose_pair(k[0, :, hs, :].reshape(S, 128))
        qk_new = np.ascontiguousarray(np.stack([qT, qTs, kT, kTs])).astype(bf16)
        in_maps.append({
            "kt_old": kt_old, "v_all": v_all, "qk_new": qk_new,
            "cs_t": cs_t,
        })
    return in_maps


def get_nc(niters=1, loop=False):
    key = ("nc", niters, loop)
    if key not in _cache:
        _cache[key] = _build(niters, loop)
    return _cache[key]


def _make_runner(nc, n_cores=NCORES):
    """Reusable jitted SPMD callable (mirrors bass2jax.run_bass_via_pjrt)
    so repeat kernel() calls skip retracing/compilation."""
    import jax
    from jax.experimental.shard_map import shard_map
    from jax.sharding import Mesh, NamedSharding, PartitionSpec

    import concourse.mybir as mybir
    from concourse.bass2jax import (_bass_exec_p, install_neuronx_cc_hook,
                                    partition_id_tensor)

    install_neuronx_cc_hook()
    partition_name = (nc.partition_id_tensor.name
                      if nc.partition_id_tensor else None)
    in_names, out_names, out_avals, zero_outs = [], [], [], []
    for alloc in nc.m.functions[0].allocations:
        if not isinstance(alloc, mybir.MemoryLocationSet):
            continue
        name = alloc.memorylocations[0].name
        if alloc.kind == "ExternalInput":
            if name != partition_name:
                in_names.append(name)
        elif alloc.kind == "ExternalOutput":
            shape = tuple(alloc.tensor_shape)
            dtype = mybir.dt.np(alloc.dtype)
            out_names.append(name)
            out_avals.append(jax.core.ShapedArray(shape, dtype))
            zero_outs.append(np.zeros(shape, dtype))
    n_params = len(in_names)
    n_outs = len(out_avals)
    all_in_names = list(in_names) + out_names
    if partition_name is not None:
        all_in_names.append(partition_name)

    def _body(*args):
        operands = list(args)
        if partition_name is not None:
            operands.append(partition_id_tensor())
        return tuple(_bass_exec_p.bind(
            *operands,
            out_avals=tuple(out_avals),
            in_names=tuple(all_in_names),
            out_names=tuple(out_names),
            lowering_input_output_aliases=(),
            sim_require_finite=True,
            sim_require_nnan=True,
            nc=nc,
        ))

    devices = jax.devices()[:n_cores]
    mesh = Mesh(np.asarray(devices), ("core",))
    sharded = jax.jit(
        shard_map(_body, mesh=mesh,
                  in_specs=(PartitionSpec("core"),) * (n_params + n_outs),
                  out_specs=(PartitionSpec("core"),) * n_outs,
                  check_rep=False),
        donate_argnums=tuple(range(n_params, n_params + n_outs)),
        keep_unused=True)
    sharding = NamedSharding(mesh, PartitionSpec("core"))

    def call(in_maps):
        concat_in = [
            np.concatenate([np.asarray(in_maps[c][nm])
                            for c in range(n_cores)], axis=0)
            for nm in in_names]
        zs = [np.zeros((n_cores * z.shape[0], *z.shape[1:]), z.dtype)
              for z in zero_outs]
        args = [jax.device_put(a, sharding) for a in concat_in + zs]
        outs = sharded(*args)
        return [
            {nm: np.asarray(outs[i]).reshape(n_cores, *out_avals[i].shape)[c]
             for i, nm in enumerate(out_names)}
            for c in range(n_cores)]

    return call


def _run(in_maps, niters=1):
    from concourse.bass_utils import run_bass_kernel_spmd
    res = run_bass_kernel_spmd(get_nc(niters), in_maps,
                               core_ids=list(range(NCORES)))
    out_full = np.empty((1, S, H, D), np.float32)
    for c in range(NCORES):
        _scatter_out(out_full, res.results[c]["out"], c)
    return out_full.reshape(1, S, H * D), res


def _scatter_out(out_full, arr, c):
    # device layout [qt, h, d, tok] -> tokens qt*512 + tok
    arr = np.asarray(arr).reshape(2, 2, D, 512)
    for qt in range(2):
        for h in range(2):
            out_full[0, qt * 512:(qt + 1) * 512, 2 * c + h, :] = arr[qt, h].T


def kernel(q, k, v, cache_k, cache_v, freqs_cos, freqs_sin):
    in_maps = _prep_inputs(q, k, v, cache_k, cache_v, freqs_cos, freqs_sin)
    try:
        if "runner" not in _cache:
            _cache["runner"] = _make_runner(get_nc(1))
        results = _cache["runner"](in_maps)
    except Exception:
        out, _ = _run(in_maps)
        return out
    out_full = np.empty((1, S, H, D), np.float32)
    for c in range(NCORES):
        _scatter_out(out_full, results[c]["out"], c)
    return out_full.reshape(1, S, H * D)

